# revision 3
# baseline (speedup 1.0000x reference)
"""GraphSAGE mean-aggregation kernel for one TRN2 chip (8 NeuronCores).

Reference computation (see problem):
    h    = feat @ w_neigh.T                      [N, 128]
    msg  = h[src]                                [E, 128]
    agg  = segment_sum(msg, dst, N)              [N, 128]
    deg  = segment_sum(ones, dst, N)
    out  = agg / max(deg, 1) + bias

Distribution (dst-sharded graph parallel):
  - Nodes sharded 8 x 12500. Core i owns dst rows [i*12500, (i+1)*12500).
  - Each core computes h for its own node shard (TensorE matmul, bf16),
    writes it to DRAM in 4 row-slices; each slice is AllGather'd separately
    so gathers on early slices overlap the remaining collective traffic.
    The h_full table is therefore PERMUTED: slice q holds rows
    [8*O[q], 8*O[q+1]) = concat over cores of their local rows [O[q],O[q+1]).
  - Each core holds the edges whose dst is in its shard.  Edge messages are
    fetched with gpsimd.dma_gather (256B rows) from the h table; the
    segment-sum runs on TensorE as one-hot matmuls accumulating in PSUM
    (dst windows of 128 nodes); epilogue fuses mean (per-partition scale)
    and bias into the PSUM->SBUF copy.
  - src indices must fit int16 for dma_gather; the table slices (<=25600
    rows) give 4 bucket views.

SPMD: all 8 cores run one instruction stream, so the chunk schedule
(C_wq = chunks per (window, bucket)) is the max over cores; the per-core
node->window assignment is chosen by a 4-dim balancing greedy so counts
pack chunks of 128 almost exactly (pad gather idx 0, pad one-hot slot -1).
"""

import sys

sys.path.insert(0, "/opt/trn_rl_repo")

import numpy as np

N_NODES = 100000
N_CORES = 8
SHARD = N_NODES // N_CORES  # 12500
D_IN = 256
D_OUT = 128
NW = (SHARD + 127) // 128  # 98 dst windows per core (last has 84 slots)
O = [0, 3200, 6400, 9600, 12500]  # h_loc row-slice boundaries (25/25/25/23 win)
NQ = 4
WPB = 4  # windows per batch
NB = (NW + WPB - 1) // WPB  # 25 batches of 4 windows (98 = 24*4+2)
SHARD_PAD = NW * 128  # 12544

_cache = {}


def _assign_windows(w4, deg):
    """Assign 12500 nodes to 98 windows (<=128 each), balancing the 4-dim
    per-bucket edge counts.  w4: [SHARD, 4] edge counts per node per bucket."""
    tot = w4.sum(axis=1)
    order = np.argsort(-tot, kind="stable")
    loads = np.zeros((NW, NQ))
    cnt = np.zeros(NW, np.int64)
    cap = np.full(NW, 128, np.int64)
    cap[NW - 1] = SHARD - (NW - 1) * 128  # 84
    Eq = w4.sum(axis=0).astype(np.float64)  # per-bucket totals
    T = Eq / NW  # target per (w, q)
    assign = np.empty(SHARD, np.int64)
    for n in order:
        x = w4[n]
        cand = loads + x
        over = np.maximum(cand - T, 0.0)
        score = (over * over).sum(axis=1) + 1e-4 * cand.sum(axis=1)
        score[cnt >= cap] = np.inf
        w = int(np.argmin(score))
        assign[n] = w
        loads[w] += x
        cnt[w] += 1
    return assign


def _preprocess(feat, w_neigh, bias, src, dst):
    src = np.asarray(src).astype(np.int64)
    dst = np.asarray(dst).astype(np.int64)
    feat = np.asarray(feat, np.float32)

    Ob = np.asarray(O)
    sz = Ob[1:] - Ob[:-1]  # slice sizes per core
    # table position of global node s (core i, local m): bucket qq = slice of m
    # bucket-local index = i*sz[qq] + (m - O[qq])
    s_core = src // SHARD
    s_m = src % SHARD
    s_q = np.searchsorted(Ob[1:], s_m, side="right")  # bucket of each edge src
    s_loc = s_core * sz[s_q] + (s_m - Ob[s_q])  # idx within bucket view

    core_of = dst // SHARD
    counts = np.zeros((N_CORES, NW * NQ), np.int64)
    per_core_raw = []
    perms = []  # per core: node -> padded out row (w*128 + slot)
    for i in range(N_CORES):
        m = core_of == i
        ed = dst[m] - i * SHARD
        eq = s_q[m]
        el = s_loc[m]
        # 4-dim balanced window assignment
        w4 = np.zeros((SHARD, NQ), np.int64)
        np.add.at(w4, (ed, eq), 1)
        assign_w = _assign_windows(w4, None)
        # slot within window
        order_by_w = np.argsort(assign_w, kind="stable")
        slot_of = np.empty(SHARD, np.int64)
        wcnt = np.bincount(assign_w, minlength=NW)
        woff = np.concatenate([[0], np.cumsum(wcnt)])
        slot_of[order_by_w] = np.arange(SHARD) - np.repeat(woff[:-1], wcnt)
        perms.append(assign_w * 128 + slot_of)

        w = assign_w[ed]
        sl_v = slot_of[ed]
        key = w * NQ + eq
        counts[i] = np.bincount(key, minlength=NW * NQ)
        order = np.argsort(key, kind="stable")
        per_core_raw.append((el[order], sl_v[order], np.concatenate([[0], np.cumsum(counts[i])])))

    # shared schedule: chunks per (window, bucket)
    cmax = counts.max(axis=0).reshape(NW, NQ)
    C_wq = (cmax + 127) // 128  # [NW, NQ]

    # batch layout: for each batch b, chunk order is (q, w, j)
    batches = []  # list of dicts
    ch_base = 0
    for b in range(NB):
        ws = list(range(b * WPB, min((b + 1) * WPB, NW)))
        runs = []  # per q: (qoff_chunks, run_chunks, [(w, woff_chunks, C)])
        off = 0
        for q in range(NQ):
            qoff = off
            wl = []
            for w in ws:
                c = int(C_wq[w, q])
                if c > 0:
                    wl.append((w, off - qoff, c))
                off += c
            runs.append((qoff, off - qoff, wl))
        batches.append({"ws": ws, "runs": runs, "ch_base": ch_base, "nch": off})
        ch_base += off
    totch = ch_base

    # per-core edge streams in schedule order
    import ml_dtypes

    in_maps = []
    wT = np.ascontiguousarray(w_neigh.T).astype(np.float32)  # [256, 128]
    bias_bc = np.tile(np.asarray(bias, np.float32)[None, :], (128, 1))

    for i in range(N_CORES):
        es, sl, goff = per_core_raw[i]
        gidx_stream = np.zeros(totch * 128, np.int16)
        slot_stream = np.full(totch * 128, -1.0, np.float32)
        for b in batches:
            for q in range(NQ):
                qoff, _, wl = b["runs"][q]
                for (w, woff, c) in wl:
                    g = w * NQ + q
                    n = goff[g + 1] - goff[g]
                    base = (b["ch_base"] + qoff + woff) * 128
                    gidx_stream[base : base + n] = es[goff[g] : goff[g + 1]]
                    slot_stream[base : base + n] = sl[goff[g] : goff[g + 1]]
        # wrap gather indices per (b, q) call
        wrap_cols = []
        for b in batches:
            for q in range(NQ):
                qoff, rq, _ = b["runs"][q]
                if rq == 0:
                    continue
                beg = (b["ch_base"] + qoff) * 128
                seg = gidx_stream[beg : beg + rq * 128]
                wrap_cols.append(seg.reshape(-1, 16).T)  # [16, rq*8]
        gw = np.tile(np.hstack(wrap_cols), (8, 1))  # [128, totch*8]
        # host-built one-hot (fp8): oh[p, P*128 + slot] = 1 for edge (P, p)
        oh_host = np.zeros((128, totch * 128), np.float32)
        e_all = np.arange(totch * 128)
        valid = slot_stream >= 0
        ev = e_all[valid]
        oh_host[ev % 128, (ev // 128) * 128 + slot_stream[valid].astype(np.int64)] = 1.0
        oh_host = oh_host.astype(ml_dtypes.float8_e4m3)

        ed_full = dst[core_of == i] - i * SHARD
        deg = np.bincount(ed_full, minlength=SHARD)
        recip = np.ones(SHARD_PAD, np.float32)
        recip[perms[i]] = (1.0 / np.maximum(deg, 1)).astype(np.float32)
        recip = np.ascontiguousarray(recip.reshape(NW, 128).T)  # [128, NW]

        featT = np.ascontiguousarray(feat[i * SHARD : (i + 1) * SHARD].T).astype(
            ml_dtypes.bfloat16
        )  # [256, 12500] bf16

        in_maps.append(
            {
                "featT": featT,
                "wT": wT,
                "bias_bc": bias_bc,
                "recip": recip,
                "gidx": np.ascontiguousarray(gw),
                "oh": oh_host,
            }
        )
    return in_maps, {"batches": batches, "totch": totch, "C_wq": C_wq, "perms": perms}


def _build(sched):
    import os

    from concourse import bacc, mybir, tile

    max_batches = int(os.environ.get("K_MAX_BATCHES", "9999"))
    skip_ag = os.environ.get("K_SKIP_AG", "0") == "1"
    skip_gather = os.environ.get("K_SKIP_GATHER", "0") == "1"
    no_packet = os.environ.get("K_NO_PACKET", "0") == "1"

    batches = sched["batches"]
    totch = sched["totch"]
    f32 = mybir.dt.float32
    bf16 = mybir.dt.bfloat16
    i16 = mybir.dt.int16

    nc = bacc.Bacc(num_devices=N_CORES, num_swdge_queues=4, dynamic_dma_scratch_size=32768)
    featT = nc.dram_tensor("featT", [D_IN, SHARD], bf16, kind="ExternalInput")
    wT = nc.dram_tensor("wT", [D_IN, D_OUT], f32, kind="ExternalInput")
    bias_bc = nc.dram_tensor("bias_bc", [128, D_OUT], f32, kind="ExternalInput")
    recip_in = nc.dram_tensor("recip", [128, NW], f32, kind="ExternalInput")
    gidx_in = nc.dram_tensor("gidx", [128, totch * 8], i16, kind="ExternalInput")
    oh_in = nc.dram_tensor("oh", [128, totch * 128], mybir.dt.float8e4, kind="ExternalInput")
    out = nc.dram_tensor("out", [SHARD_PAD, D_OUT], f32, kind="ExternalOutput")

    # AG slice q covers h_loc rows [O[q], O[q+1]) -> h_full rows 8*O[q]..
    ag_after_window = {}  # window idx -> slice q
    for q in range(NQ):
        ag_after_window[(O[q + 1] + 127) // 128 - 1] = q

    with tile.TileContext(nc) as tc:
        with (
            tc.tile_pool(name="dram", bufs=1, space="DRAM") as dram,
            tc.tile_pool(name="const", bufs=1) as constp,
            tc.tile_pool(name="ft", bufs=1) as ftp,
            tc.tile_pool(name="sb", bufs=2) as sb,
            tc.tile_pool(name="hbp", bufs=3) as hbp,
            tc.tile_pool(name="psA", bufs=2, space="PSUM") as psA,
            tc.tile_pool(name="psB", bufs=4, space="PSUM") as psB,
        ):
            h_loc = dram.tile([SHARD, D_OUT], bf16)
            h_full = dram.tile([8 * SHARD, D_OUT], bf16, addr_space="Shared")

            # constants
            wt = constp.tile([128, 2, D_OUT], bf16)
            nc.gpsimd.dma_start(out=wt[:], in_=wT[:, :].rearrange("(a k) n -> k a n", k=128))
            biast = constp.tile([128, D_OUT], f32)
            nc.sync.dma_start(out=biast[:], in_=bias_bc[:, :])
            recip = constp.tile([128, NW], f32)
            nc.sync.dma_start(out=recip[:], in_=recip_in[:, :])

            # ---- prologue: h_loc = (feat @ w.T) in bf16, AG per slice ----
            ft0 = ftp.tile([128, SHARD], bf16)
            nc.sync.dma_start(out=ft0[:], in_=featT[0:128, :])
            ft1 = ftp.tile([128, SHARD], bf16)
            nc.sync.dma_start(out=ft1[:], in_=featT[128:256, :])
            for t in range(NW):
                lo = t * 128
                cnt = min(128, SHARD - lo)
                ph = psA.tile([128, D_OUT], f32, space="PSUM")
                nc.tensor.matmul(ph[:cnt, :], lhsT=ft0[:, lo : lo + cnt], rhs=wt[:, 0, :], start=True, stop=False)
                nc.tensor.matmul(ph[:cnt, :], lhsT=ft1[:, lo : lo + cnt], rhs=wt[:, 1, :], start=False, stop=True)
                hb = hbp.tile([128, D_OUT], bf16)
                nc.scalar.activation(hb[:cnt, :], ph[:cnt, :], mybir.ActivationFunctionType.Copy)
                nc.sync.dma_start(out=h_loc[lo : lo + cnt, :], in_=hb[:cnt, :])
                q = ag_after_window.get(t)
                if q is not None:
                    if skip_ag:
                        nc.sync.dma_start(
                            out=h_full[8 * O[q] : 8 * O[q] + (O[q + 1] - O[q]), :],
                            in_=h_loc[O[q] : O[q + 1], :],
                        )
                    else:
                        nc.gpsimd.collective_compute(
                            "AllGather",
                            mybir.AluOpType.bypass,
                            replica_groups=[list(range(N_CORES))],
                            ins=[h_loc[O[q] : O[q + 1], :].opt()],
                            outs=[h_full[8 * O[q] : 8 * O[q + 1], :].opt()],
                        )

            # ---- main loop ----
            qrr = [0]
            for b in batches[:max_batches]:
                nch = b["nch"]
                cb = b["ch_base"]
                gi = sb.tile([128, nch * 8], i16, tag="gi")
                nc.sync.dma_start(out=gi[:], in_=gidx_in[:, cb * 8 : (cb + nch) * 8])
                ohb = sb.tile([128, nch, 128], mybir.dt.float8e4, tag="ohb")
                nc.sync.dma_start(out=ohb[:], in_=oh_in[:, cb * 128 : (cb + nch) * 128])
                msg = sb.tile([128, nch, D_OUT], bf16, tag="msg", bufs=3)
                for q in range(NQ):
                    qoff, rq, _ = b["runs"][q]
                    if rq == 0:
                        continue
                    if skip_gather:
                        continue
                    hview = h_full[8 * O[q] : 8 * O[q + 1], :]
                    if no_packet:
                        nc.gpsimd.dma_gather(
                            msg[:, qoff : qoff + rq, :],
                            hview,
                            gi[:, qoff * 8 : (qoff + rq) * 8],
                            num_idxs=rq * 128,
                            num_idxs_reg=rq * 128,
                            elem_size=D_OUT,
                            queue_num=qrr[0] % 4,
                            single_packet=False,
                        )
                        qrr[0] += 1
                        continue
                    # single_packet=True requires <= ~64 descriptors per SDMA
                    # engine per packet -> split into sub-calls of 8 chunks
                    # (1024 idxs).
                    for s0 in range(0, rq, 8):
                        sc = min(8, rq - s0)
                        o = qoff + s0
                        nc.gpsimd.dma_gather(
                            msg[:, o : o + sc, :],
                            hview,
                            gi[:, o * 8 : (o + sc) * 8],
                            num_idxs=sc * 128,
                            num_idxs_reg=sc * 128,
                            elem_size=D_OUT,
                            queue_num=qrr[0] % 4,
                            single_packet=True,
                        )
                        qrr[0] += 1
                ot = sb.tile([128, len(b["ws"]), D_OUT], f32, tag="ot")
                for wi, w in enumerate(b["ws"]):
                    runs_w = []
                    for q in range(NQ):
                        qoff, _, wl = b["runs"][q]
                        for (ww, woff, c) in wl:
                            if ww == w:
                                runs_w.append((qoff + woff, c))
                    total_c = sum(c for _, c in runs_w)
                    assert total_c > 0
                    pw = psB.tile([128, D_OUT], f32, space="PSUM", tag="pw")
                    done = 0
                    for (base, c) in runs_w:
                        for j in range(c):
                            nc.tensor.matmul(
                                pw[:, :],
                                lhsT=ohb[:, base + j, :],
                                rhs=msg[:, base + j, :],
                                start=(done == 0),
                                stop=(done == total_c - 1),
                            )
                            done += 1
                    nc.scalar.activation(
                        ot[:, wi, :], pw[:, :], mybir.ActivationFunctionType.Copy,
                        scale=recip[:, w : w + 1],
                    )
                    nc.vector.tensor_add(ot[:, wi, :], ot[:, wi, :], biast[:])
                nw_b = len(b["ws"])
                w0 = b["ws"][0]
                nc.sync.dma_start(
                    out=out[w0 * 128 : w0 * 128 + nw_b * 128, :].rearrange(
                        "(c p) f -> p c f", p=128
                    ),
                    in_=ot[:],
                )

    nc.finalize()
    return nc


def _run(inputs, trace=False):
    from concourse.bass_utils import run_bass_kernel_spmd

    key = "k"
    in_maps, sched = _preprocess(
        inputs["feat"], inputs["w_neigh"], inputs["bias"], inputs["src"], inputs["dst"]
    )
    if key not in _cache:
        _cache[key] = _build(sched)
    nc = _cache[key]
    res = run_bass_kernel_spmd(nc, in_maps, core_ids=list(range(N_CORES)), trace=trace)
    outs = [res.results[i]["out"][sched["perms"][i]] for i in range(N_CORES)]
    full = np.concatenate(outs, axis=0)
    return full, res


def kernel(**inputs):
    full, _ = _run(inputs, trace=False)
    return full


# revision 10
# speedup vs baseline: 1.5750x; 1.5750x over previous
"""GraphSAGE mean-aggregation kernel for one TRN2 chip (8 NeuronCores).

Reference computation (see problem):
    h    = feat @ w_neigh.T                      [N, 128]
    msg  = h[src]                                [E, 128]
    agg  = segment_sum(msg, dst, N)              [N, 128]
    deg  = segment_sum(ones, dst, N)
    out  = agg / max(deg, 1) + bias

Distribution (dst-sharded graph parallel):
  - Nodes sharded 8 x 12500. Core i owns dst rows [i*12500, (i+1)*12500).
  - Each core computes h for its own node shard (TensorE matmul, bf16),
    writes it to DRAM in 4 row-slices; each slice is AllGather'd separately
    so gathers on early slices overlap the remaining collective traffic.
    The h_full table is therefore PERMUTED: slice q holds rows
    [8*O[q], 8*O[q+1]) = concat over cores of their local rows [O[q],O[q+1]).
  - Each core holds the edges whose dst is in its shard.  Edge messages are
    fetched with gpsimd.dma_gather (256B rows) from the h table; the
    segment-sum runs on TensorE as one-hot matmuls accumulating in PSUM
    (dst windows of 128 nodes); epilogue fuses mean (per-partition scale)
    and bias into the PSUM->SBUF copy.
  - src indices must fit int16 for dma_gather; the table slices (<=25600
    rows) give 4 bucket views.

SPMD: all 8 cores run one instruction stream, so the chunk schedule
(C_wq = chunks per (window, bucket)) is the max over cores; the per-core
node->window assignment is chosen by a 4-dim balancing greedy so counts
pack chunks of 128 almost exactly (pad gather idx 0, pad one-hot slot -1).
"""

import sys

sys.path.insert(0, "/opt/trn_rl_repo")

import numpy as np

N_NODES = 100000
N_CORES = 8
SHARD = N_NODES // N_CORES  # 12500
D_IN = 256
D_OUT = 128
NT = (SHARD + 127) // 128  # 98 prologue tiles over h_loc rows
NW = 99  # dst aggregation windows (12672 slots; slack for 4-dim packing)
O = [0, 3125, 6250, 9375, 12500]  # h_loc row-slice boundaries (equal AG slices)
NQ = 4
WPB = 4  # windows per batch
NB = (NW + WPB - 1) // WPB  # 25 batches of 4 windows
SHARD_PAD = NW * 128  # 12672

_cache = {}


def _assign_windows(w4, deg):
    """Assign 12500 nodes to 98 windows (<=128 each), balancing the 4-dim
    per-bucket edge counts.  w4: [SHARD, 4] edge counts per node per bucket."""
    tot = w4.sum(axis=1)
    order = np.argsort(-tot, kind="stable")
    loads = np.zeros((NW, NQ))
    cnt = np.zeros(NW, np.int64)
    cap = np.full(NW, 128, np.int64)
    CHUNK_CAP = 512.0  # 4 chunks of 128: stay under to keep C_wq == 4
    T = 500.0
    assign = np.empty(SHARD, np.int64)
    for n in order:
        x = w4[n]
        cand = loads + x
        over_cap = np.maximum(cand - CHUNK_CAP, 0.0)
        over_t = np.maximum(cand - T, 0.0)
        score = 1e6 * over_cap.sum(axis=1) + (over_t * over_t).sum(axis=1) + 1e-4 * cand.sum(axis=1)
        score[cnt >= cap] = np.inf
        w = int(np.argmin(score))
        assign[n] = w
        loads[w] += x
        cnt[w] += 1
    return assign


def _preprocess(feat, w_neigh, bias, src, dst):
    src = np.asarray(src).astype(np.int64)
    dst = np.asarray(dst).astype(np.int64)
    feat = np.asarray(feat, np.float32)

    Ob = np.asarray(O)
    sz = Ob[1:] - Ob[:-1]  # slice sizes per core
    # table position of global node s (core i, local m): bucket qq = slice of m
    # bucket-local index = i*sz[qq] + (m - O[qq])
    s_core = src // SHARD
    s_m = src % SHARD
    s_q = np.searchsorted(Ob[1:], s_m, side="right")  # bucket of each edge src
    s_loc = s_core * sz[s_q] + (s_m - Ob[s_q])  # idx within bucket view

    core_of = dst // SHARD
    counts = np.zeros((N_CORES, NW * NQ), np.int64)
    per_core_raw = []
    perms = []  # per core: node -> padded out row (w*128 + slot)
    for i in range(N_CORES):
        m = core_of == i
        ed = dst[m] - i * SHARD
        eq = s_q[m]
        el = s_loc[m]
        # 4-dim balanced window assignment
        w4 = np.zeros((SHARD, NQ), np.int64)
        np.add.at(w4, (ed, eq), 1)
        assign_w = _assign_windows(w4, None)
        # slot within window
        order_by_w = np.argsort(assign_w, kind="stable")
        slot_of = np.empty(SHARD, np.int64)
        wcnt = np.bincount(assign_w, minlength=NW)
        woff = np.concatenate([[0], np.cumsum(wcnt)])
        slot_of[order_by_w] = np.arange(SHARD) - np.repeat(woff[:-1], wcnt)
        perms.append(assign_w * 128 + slot_of)

        w = assign_w[ed]
        sl_v = slot_of[ed]
        key = w * NQ + eq
        counts[i] = np.bincount(key, minlength=NW * NQ)
        order = np.argsort(key, kind="stable")
        per_core_raw.append((el[order], sl_v[order], np.concatenate([[0], np.cumsum(counts[i])])))

    # shared schedule: chunks per (window, bucket)
    cmax = counts.max(axis=0).reshape(NW, NQ)
    C_wq = (cmax + 127) // 128  # [NW, NQ]

    # batch layout: for each batch b, chunk order is (q, w, j)
    batches = []  # list of dicts
    ch_base = 0
    for b in range(NB):
        ws = list(range(b * WPB, min((b + 1) * WPB, NW)))
        runs = []  # per q: (qoff_chunks, run_chunks, [(w, woff_chunks, C)])
        off = 0
        for q in range(NQ):
            qoff = off
            wl = []
            for w in ws:
                c = int(C_wq[w, q])
                if c > 0:
                    wl.append((w, off - qoff, c))
                off += c
            runs.append((qoff, off - qoff, wl))
        batches.append({"ws": ws, "runs": runs, "ch_base": ch_base, "nch": off})
        ch_base += off
    totch = ch_base

    # per-core edge streams in schedule order
    import ml_dtypes

    in_maps = []
    wT = np.ascontiguousarray(w_neigh.T).astype(np.float32)  # [256, 128]
    bias_bc = np.tile(np.asarray(bias, np.float32)[None, :], (128, 1))

    for i in range(N_CORES):
        es, sl, goff = per_core_raw[i]
        gidx_stream = np.zeros(totch * 128, np.int16)
        slot_stream = np.full(totch * 128, -1.0, np.float32)
        for b in batches:
            for q in range(NQ):
                qoff, _, wl = b["runs"][q]
                for (w, woff, c) in wl:
                    g = w * NQ + q
                    n = goff[g + 1] - goff[g]
                    base = (b["ch_base"] + qoff + woff) * 128
                    gidx_stream[base : base + n] = es[goff[g] : goff[g + 1]]
                    slot_stream[base : base + n] = sl[goff[g] : goff[g + 1]]
        # wrap gather indices per (b, q) call
        wrap_cols = []
        for b in batches:
            for q in range(NQ):
                qoff, rq, _ = b["runs"][q]
                if rq == 0:
                    continue
                beg = (b["ch_base"] + qoff) * 128
                seg = gidx_stream[beg : beg + rq * 128]
                wrap_cols.append(seg.reshape(-1, 16).T)  # [16, rq*8]
        gw = np.tile(np.hstack(wrap_cols), (8, 1))  # [128, totch*8]
        # host-built one-hot (fp8): oh[p, P*128 + slot] = 1 for edge (P, p)
        oh_host = np.zeros((128, totch * 128), np.float32)
        e_all = np.arange(totch * 128)
        valid = slot_stream >= 0
        ev = e_all[valid]
        oh_host[ev % 128, (ev // 128) * 128 + slot_stream[valid].astype(np.int64)] = 1.0
        oh_host = oh_host.astype(ml_dtypes.float8_e4m3)

        ed_full = dst[core_of == i] - i * SHARD
        deg = np.bincount(ed_full, minlength=SHARD)
        recip = np.ones(SHARD_PAD, np.float32)
        recip[perms[i]] = (1.0 / np.maximum(deg, 1)).astype(np.float32)
        recip = np.ascontiguousarray(recip.reshape(NW, 128).T)  # [128, NW]

        featT = np.ascontiguousarray(feat[i * SHARD : (i + 1) * SHARD].T).astype(
            ml_dtypes.bfloat16
        )  # [256, 12500] bf16

        in_maps.append(
            {
                "featT": featT,
                "wT": wT,
                "bias_bc": bias_bc,
                "recip": recip,
                "gidx": np.ascontiguousarray(gw),
                "oh": oh_host,
            }
        )
    return in_maps, {"batches": batches, "totch": totch, "C_wq": C_wq, "perms": perms}


def _build(sched):
    import os

    from concourse import bacc, mybir, tile

    max_batches = int(os.environ.get("K_MAX_BATCHES", "9999"))
    skip_ag = os.environ.get("K_SKIP_AG", "0") == "1"
    skip_gather = os.environ.get("K_SKIP_GATHER", "0") == "1"
    no_packet = os.environ.get("K_NO_PACKET", "0") == "1"

    batches = sched["batches"]
    totch = sched["totch"]
    f32 = mybir.dt.float32
    bf16 = mybir.dt.bfloat16
    i16 = mybir.dt.int16

    nc = bacc.Bacc(num_devices=N_CORES, num_swdge_queues=4, dynamic_dma_scratch_size=32768)
    featT = nc.dram_tensor("featT", [D_IN, SHARD], bf16, kind="ExternalInput")
    wT = nc.dram_tensor("wT", [D_IN, D_OUT], f32, kind="ExternalInput")
    bias_bc = nc.dram_tensor("bias_bc", [128, D_OUT], f32, kind="ExternalInput")
    recip_in = nc.dram_tensor("recip", [128, NW], f32, kind="ExternalInput")
    gidx_in = nc.dram_tensor("gidx", [128, totch * 8], i16, kind="ExternalInput")
    oh_in = nc.dram_tensor("oh", [128, totch * 128], mybir.dt.float8e4, kind="ExternalInput")
    out = nc.dram_tensor("out", [SHARD_PAD, D_OUT], f32, kind="ExternalOutput")

    # AG slice q covers h_loc rows [O[q], O[q+1]) -> h_full rows 8*O[q]..
    ag_after_window = {}  # window idx -> slice q
    for q in range(NQ):
        ag_after_window[(O[q + 1] + 127) // 128 - 1] = q

    with tile.TileContext(nc) as tc:
        with (
            tc.tile_pool(name="dram", bufs=1, space="DRAM") as dram,
            tc.tile_pool(name="const", bufs=1) as constp,
            tc.tile_pool(name="ft", bufs=1) as ftp,
            tc.tile_pool(name="sb", bufs=2) as sb,
            tc.tile_pool(name="hbp", bufs=3) as hbp,
            tc.tile_pool(name="psA", bufs=2, space="PSUM") as psA,
            tc.tile_pool(name="psB", bufs=4, space="PSUM") as psB,
        ):
            h_loc = dram.tile([SHARD, D_OUT], bf16)
            h_slice = [
                dram.tile(
                    [8 * (O[q + 1] - O[q]), D_OUT],
                    bf16,
                    addr_space="Shared",
                    name=f"h_slice{q}",
                )
                for q in range(NQ)
            ]

            # constants
            wt = constp.tile([128, 2, D_OUT], bf16)
            nc.gpsimd.dma_start(out=wt[:], in_=wT[:, :].rearrange("(a k) n -> k a n", k=128))
            biast = constp.tile([128, D_OUT], f32)
            nc.sync.dma_start(out=biast[:], in_=bias_bc[:, :])
            recip = constp.tile([128, NW], f32)
            nc.sync.dma_start(out=recip[:], in_=recip_in[:, :])

            # ---- prologue: h_loc = (feat @ w.T) in bf16, AG per slice ----
            ft0 = ftp.tile([128, SHARD], bf16)
            nc.sync.dma_start(out=ft0[:], in_=featT[0:128, :])
            ft1 = ftp.tile([128, SHARD], bf16)
            nc.sync.dma_start(out=ft1[:], in_=featT[128:256, :])
            for t in range(NT):
                lo = t * 128
                cnt = min(128, SHARD - lo)
                ph = psA.tile([128, D_OUT], f32, space="PSUM")
                nc.tensor.matmul(ph[:cnt, :], lhsT=ft0[:, lo : lo + cnt], rhs=wt[:, 0, :], start=True, stop=False)
                nc.tensor.matmul(ph[:cnt, :], lhsT=ft1[:, lo : lo + cnt], rhs=wt[:, 1, :], start=False, stop=True)
                hb = hbp.tile([128, D_OUT], bf16)
                nc.scalar.activation(hb[:cnt, :], ph[:cnt, :], mybir.ActivationFunctionType.Copy)
                nc.sync.dma_start(out=h_loc[lo : lo + cnt, :], in_=hb[:cnt, :])
                q = ag_after_window.get(t)
                if q is not None:
                    if skip_ag:
                        nc.sync.dma_start(
                            out=h_slice[q][0 : O[q + 1] - O[q], :],
                            in_=h_loc[O[q] : O[q + 1], :],
                        )
                    else:
                        nc.gpsimd.collective_compute(
                            "AllGather",
                            mybir.AluOpType.bypass,
                            replica_groups=[list(range(N_CORES))],
                            ins=[h_loc[O[q] : O[q + 1], :].opt()],
                            outs=[h_slice[q][:].opt()],
                        )

            # ---- main loop ----
            qrr = [0]
            for b in batches[:max_batches]:
                nch = b["nch"]
                cb = b["ch_base"]
                gi = sb.tile([128, nch * 8], i16, tag="gi")
                nc.sync.dma_start(out=gi[:], in_=gidx_in[:, cb * 8 : (cb + nch) * 8])
                ohb = sb.tile([128, nch, 128], mybir.dt.float8e4, tag="ohb")
                nc.sync.dma_start(out=ohb[:], in_=oh_in[:, cb * 128 : (cb + nch) * 128])
                msg = sb.tile([128, nch, D_OUT], bf16, tag="msg", bufs=3)
                for q in range(NQ):
                    qoff, rq, _ = b["runs"][q]
                    if rq == 0:
                        continue
                    if skip_gather:
                        continue
                    hview = h_slice[q][:]
                    if no_packet:
                        nc.gpsimd.dma_gather(
                            msg[:, qoff : qoff + rq, :],
                            hview,
                            gi[:, qoff * 8 : (qoff + rq) * 8],
                            num_idxs=rq * 128,
                            num_idxs_reg=rq * 128,
                            elem_size=D_OUT,
                            queue_num=qrr[0] % 4,
                            single_packet=False,
                        )
                        qrr[0] += 1
                        continue
                    # single_packet=True requires <= ~64 descriptors per SDMA
                    # engine per packet -> split into sub-calls of 8 chunks
                    # (1024 idxs).
                    for s0 in range(0, rq, 8):
                        sc = min(8, rq - s0)
                        o = qoff + s0
                        nc.gpsimd.dma_gather(
                            msg[:, o : o + sc, :],
                            hview,
                            gi[:, o * 8 : (o + sc) * 8],
                            num_idxs=sc * 128,
                            num_idxs_reg=sc * 128,
                            elem_size=D_OUT,
                            queue_num=qrr[0] % 4,
                            single_packet=True,
                        )
                        qrr[0] += 1
                ot = sb.tile([128, len(b["ws"]), D_OUT], f32, tag="ot")
                for wi, w in enumerate(b["ws"]):
                    runs_w = []
                    for q in range(NQ):
                        qoff, _, wl = b["runs"][q]
                        for (ww, woff, c) in wl:
                            if ww == w:
                                runs_w.append((qoff + woff, c))
                    total_c = sum(c for _, c in runs_w)
                    assert total_c > 0
                    pw = psB.tile([128, D_OUT], f32, space="PSUM", tag="pw")
                    done = 0
                    for (base, c) in runs_w:
                        for j in range(c):
                            nc.tensor.matmul(
                                pw[:, :],
                                lhsT=ohb[:, base + j, :],
                                rhs=msg[:, base + j, :],
                                start=(done == 0),
                                stop=(done == total_c - 1),
                            )
                            done += 1
                    nc.scalar.activation(
                        ot[:, wi, :], pw[:, :], mybir.ActivationFunctionType.Copy,
                        scale=recip[:, w : w + 1],
                    )
                    nc.vector.tensor_add(ot[:, wi, :], ot[:, wi, :], biast[:])
                nw_b = len(b["ws"])
                w0 = b["ws"][0]
                nc.sync.dma_start(
                    out=out[w0 * 128 : w0 * 128 + nw_b * 128, :].rearrange(
                        "(c p) f -> p c f", p=128
                    ),
                    in_=ot[:],
                )

    nc.finalize()
    return nc


def _run(inputs, trace=False):
    from concourse.bass_utils import run_bass_kernel_spmd

    key = "k"
    in_maps, sched = _preprocess(
        inputs["feat"], inputs["w_neigh"], inputs["bias"], inputs["src"], inputs["dst"]
    )
    if key not in _cache:
        _cache[key] = _build(sched)
    nc = _cache[key]
    res = run_bass_kernel_spmd(nc, in_maps, core_ids=list(range(N_CORES)), trace=trace)
    outs = [res.results[i]["out"][sched["perms"][i]] for i in range(N_CORES)]
    full = np.concatenate(outs, axis=0)
    return full, res


def kernel(**inputs):
    full, _ = _run(inputs, trace=False)
    return full
